# revision 20
# baseline (speedup 1.0000x reference)
"""Trainium2 Bass kernel for nn_CrossAttention2D_ROPE (B=8, S1=4096, S2=256,
QDIM=1024, KDIM=2048, NH=16, HD=64).

Strategy: data-parallel over batch (8 cores, one batch element each). Per core
a fused pipeline in bf16 (fp32 accumulation):

  - Weights host-prepped: transposed to (in, out), per-head row mean-centering
    folded into wq/wk, RoPE expressed as elementwise C/S tables in q^T layout
    with softmax scale and q-gain folded in.
  - Projections on PE with contraction on partitions (x^T/y^T via bf16 cast +
    DMA transpose). Startup DMAs spread across the sync/scalar/vector/gpsimd
    queues so compute starts ~30us in instead of ~100us.
  - RMS-norm: sum-of-squares via ONE block-diagonal ones matmul whose output
    is already replicated across each head's 64 partitions (fp32-exact), so
    sqrt/reciprocal run at full width and no fp32 broadcast matmul is needed.
  - RoPE: qr = (q*C + pairswap(q)*S) * rstd; pairswap via DVE stream_shuffle;
    elementwise work split across DVE and GpSimd.
  - Attention per head pair: score matmuls (K=64) for the two heads of a pair
    are packed into disjoint PE row groups (concurrent); exp on ScalarE out of
    PSUM; one M=128 [v|ones] matmul produces attn@v (rows 0:64) and the
    softmax denominator (rows 64:128) in a single pass; normalize via a
    cross-partition-base reciprocal+multiply on DVE.
  - Output projection in natural layout, bias via K=1 ones-row matmuls.
"""

import os
import numpy as np
import ml_dtypes

BF = ml_dtypes.bfloat16
QDIM, KDIM, NH, HD = 1024, 2048, 16, 64
H, W, B, S2 = 64, 64, 8, 256
S1 = H * W
EPS = 1e-6
SCALE = HD ** -0.5
NQ = 4            # process s1 in quarters
S1Q = S1 // NQ    # 1024

last_exec_time_ns = None
last_trace_path = None


# ----------------------------------------------------------------- host prep
def _bf16(a):
    return np.asarray(a, np.float32).astype(BF)


def _f32(a):
    return np.asarray(a, np.float32)


def _center_rows_per_head(w):
    out = _f32(w).copy()
    for _ in range(3):
        wb = _f32(_bf16(out))
        resid = wb.reshape(NH, HD, -1).mean(axis=1, keepdims=True)
        out = wb - np.broadcast_to(resid, (NH, HD, wb.shape[-1])).reshape(wb.shape)
    return _bf16(out)


def _center_bias_per_head(b):
    b = _f32(b)
    return b - np.repeat(b.reshape(NH, HD).mean(axis=1), HD)


def _rope_tables(qn_g):
    d4 = HD // 4
    inv = 1.0 / (10000.0 ** (np.arange(0, HD // 2, 2, dtype=np.float32) / (HD // 2)))
    fh = np.outer(np.arange(H, dtype=np.float32), inv)
    fw = np.outer(np.arange(W, dtype=np.float32), inv)
    ang = np.stack([
        np.broadcast_to(fh[:, None, :], (H, W, d4)),
        np.broadcast_to(fw[None, :, :], (H, W, d4)),
    ], axis=-1).reshape(S1, HD // 2)
    cos = np.cos(ang)
    sin = np.sin(ang)
    g = _f32(qn_g)
    C = np.zeros((HD, S1), np.float32)
    S = np.zeros((HD, S1), np.float32)
    for i in range(HD // 2):
        C[2 * i] = g[2 * i] * cos[:, i] * SCALE
        C[2 * i + 1] = g[2 * i + 1] * cos[:, i] * SCALE
        S[2 * i] = -g[2 * i + 1] * sin[:, i] * SCALE
        S[2 * i + 1] = g[2 * i] * sin[:, i] * SCALE
    return _bf16(np.concatenate([C, C], 0)), _bf16(np.concatenate([S, S], 0))


def _host_tables(wq, bq, wkv, bkv, wo, bo, qn_g, qn_b, kn_g, kn_b):
    assert not np.any(_f32(qn_b)) and not np.any(_f32(kn_b)), \
        "nonzero layernorm beta not implemented"
    t = {}
    t["WqT"] = np.ascontiguousarray(_center_rows_per_head(wq).T)
    t["bq_c"] = _center_bias_per_head(bq).reshape(QDIM, 1)
    t["WkT"] = np.ascontiguousarray(_center_rows_per_head(_f32(wkv)[0:QDIM]).T)
    t["bk_c"] = _center_bias_per_head(_f32(bkv)[0:QDIM]).reshape(QDIM, 1)
    t["WvT"] = np.ascontiguousarray(_bf16(_f32(wkv)[QDIM:]).T)
    t["bv"] = _bf16(_f32(bkv)[QDIM:]).reshape(1, QDIM)
    t["WoT"] = np.ascontiguousarray(_bf16(wo).T)
    t["bo"] = _bf16(bo).reshape(1, QDIM)
    t["CT"], t["ST"] = _rope_tables(qn_g)
    t["gk_col"] = np.tile(_f32(kn_g), 2).reshape(128, 1)
    # block-diagonal ones: col m sums the 64-partition strip containing m,
    # so lhsT=onesblk replicates each strip's partition-sum across the strip
    blk = np.zeros((128, 128), np.float32)
    blk[0:64, 0:64] = 1.0
    blk[64:128, 64:128] = 1.0
    t["onesblk"] = _bf16(blk)
    t["ones1"] = _bf16(np.ones((1, 128), np.float32))
    return t


# ------------------------------------------------------------- bass program
_PROGRAM = None


def _build_program():
    import concourse.bass as bass
    import concourse.bacc as bacc
    import concourse.mybir as mybir
    import concourse.tile as tile
    from contextlib import ExitStack

    bfd = mybir.dt.bfloat16
    f32d = mybir.dt.float32
    AF = mybir.ActivationFunctionType
    AO = mybir.AluOpType

    nc = bacc.Bacc("TRN2", target_bir_lowering=False, debug=False)

    def din(name, shape, dt):
        return nc.dram_tensor(name, shape, dt, kind="ExternalInput").ap()

    x_d = din("x", [S1, QDIM], f32d)
    y_d = din("y", [S2, KDIM], f32d)
    wqT_d = din("WqT", [QDIM, QDIM], bfd)
    wkT_d = din("WkT", [KDIM, QDIM], bfd)
    wvT_d = din("WvT", [KDIM, QDIM], bfd)
    woT_d = din("WoT", [QDIM, QDIM], bfd)
    bq_d = din("bq_c", [QDIM, 1], f32d)
    bk_d = din("bk_c", [QDIM, 1], f32d)
    bv_d = din("bv", [1, QDIM], bfd)
    bo_d = din("bo", [1, QDIM], bfd)
    ct_d = din("CT", [128, S1], bfd)
    st_d = din("ST", [128, S1], bfd)
    gk_d = din("gk_col", [128, 1], f32d)
    onesblk_d = din("onesblk", [128, 128], bfd)
    ones1_d = din("ones1", [1, 128], bfd)
    out_d = nc.dram_tensor("out", [S1, QDIM], f32d, kind="ExternalOutput").ap()
    xbf = nc.dram_tensor("xbf", [S1, QDIM], bfd).ap()
    ybf = nc.dram_tensor("ybf", [S2, KDIM], bfd).ap()

    swap_mask = []
    for g in range(16):
        swap_mask += [2 * g + 1, 2 * g]

    with tile.TileContext(nc) as tc, ExitStack() as ctx:
        const = ctx.enter_context(tc.tile_pool(name="const", bufs=1))
        persist = ctx.enter_context(tc.tile_pool(name="persist", bufs=1))
        # kernel-wide PSUM plan (8 banks total):
        #   psMain "mm"  2x[128,512] = 2 banks (proj accumulators + norm reps)
        #   psSc  pscA/pscB 1x[128,1024] each = 4 banks (scores; V-proj accums)
        #   psPb  pbA/pbB  1x[128,512] each = 2 banks (attn@v + denominator)
        psMain = ctx.enter_context(tc.tile_pool(name="psMain", bufs=2, space="PSUM"))
        psSc = ctx.enter_context(tc.tile_pool(name="psSc", bufs=1, space="PSUM"))
        psPb = ctx.enter_context(tc.tile_pool(name="psPb", bufs=1, space="PSUM"))
        qwork = ctx.enter_context(tc.tile_pool(name="qwork", bufs=2))
        awork = ctx.enter_context(tc.tile_pool(name="awork", bufs=2))
        owork = ctx.enter_context(tc.tile_pool(name="owork", bufs=2))

        # ---- constants (sync queue; small)
        bq8 = const.tile([128, 8], f32d)
        nc.sync.dma_start(out=bq8[:], in_=bq_d.rearrange("(m p) o -> p (m o)", p=128))
        bk8 = const.tile([128, 8], f32d)
        nc.sync.dma_start(out=bk8[:], in_=bk_d.rearrange("(m p) o -> p (m o)", p=128))
        gk_t = const.tile([128, 1], f32d)
        nc.sync.dma_start(out=gk_t[:], in_=gk_d[:])
        bv_t = const.tile([1, QDIM], bfd)
        nc.sync.dma_start(out=bv_t[:], in_=bv_d[:])
        bo_t = const.tile([1, QDIM], bfd)
        nc.sync.dma_start(out=bo_t[:], in_=bo_d[:])
        onesblk_t = const.tile([128, 128], bfd)
        nc.sync.dma_start(out=onesblk_t[:], in_=onesblk_d[:])
        ones1_t = const.tile([1, 128], bfd)
        nc.sync.dma_start(out=ones1_t[:], in_=ones1_d[:])
        ct_t = const.tile([128, S1], bfd)
        nc.sync.dma_start(out=ct_t[:], in_=ct_d[:])
        st_t = const.tile([128, S1], bfd)
        nc.sync.dma_start(out=st_t[:], in_=st_d[:])
        eps_t = const.tile([128, 1], f32d)
        nc.vector.memset(eps_t[:, :], EPS)

        # ---- persistent activations / weights
        kn_t = persist.tile([128, 8, S2], bfd)       # normalized k^T
        va_t = persist.tile([128, 2, NH, 128], bfd)  # [v_h | ones] per s2-tile
        wq_t = persist.tile([128, 8, QDIM], bfd)
        wo_t = persist.tile([128, 8, QDIM], bfd)
        qr_t = persist.tile([128, 8, S1Q], bfd)      # per-quarter roped q^T
        aT_t = persist.tile([128, 8, S1Q], bfd)      # per-quarter attn out^T
        xt_t = persist.tile([128, 8, S1Q], bfd)      # per-quarter x^T

        # ---- dtype casts (SWDGE on the gpsimd queue); quarter 0 first, in
        # token halves so the first projection matmuls can start early
        nc.gpsimd.dma_start(out=xbf[0:512, :], in_=x_d[0:512, :])
        nc.gpsimd.dma_start(out=xbf[512:S1Q, :], in_=x_d[512:S1Q, :])
        nc.gpsimd.dma_start(out=ybf[:], in_=y_d[:])
        for qq in range(1, NQ):
            nc.gpsimd.dma_start(
                out=xbf[qq * S1Q:(qq + 1) * S1Q, :], in_=x_d[qq * S1Q:(qq + 1) * S1Q, :]
            )
        # wq on scalar queue (needed first), then quarter-0 x^T transposes
        nc.scalar.dma_start(
            out=wq_t[:], in_=wqT_d.rearrange("(t p) c -> p t c", p=128)
        )
        for half in range(2):
            hs, he = 512 * half, 512 * (half + 1)
            for k in range(8):
                nc.scalar.dma_start_transpose(
                    out=xt_t[:, k, hs:he], in_=xbf[hs:he, 128 * k:128 * (k + 1)]
                )

        # ================= per-quarter phases =================
        def q_phase(qq):
            qoff = qq * S1Q
            for m in range(8):
                qt = qwork.tile([128, S1Q], bfd, tag="qt")
                rstd = qwork.tile([128, S1Q], f32d, tag="rstd")
                for n in range(2):
                    sl = slice(512 * n, 512 * (n + 1))
                    psq = psMain.tile([128, 512], f32d, tag="mm")
                    for k in range(8):
                        nc.tensor.matmul(
                            psq[:], wq_t[:, k, 128 * m:128 * (m + 1)],
                            xt_t[:, k, sl], start=(k == 0), stop=(k == 7),
                        )
                    nc.scalar.activation(out=qt[:, sl], in_=psq[:], func=AF.Identity,
                                         bias=bq8[:, m:m + 1], scale=1.0)
                    q2 = qwork.tile([128, 512], bfd, tag="q2")
                    nc.vector.tensor_mul(q2[:], qt[:, sl], qt[:, sl])
                    ssq = psMain.tile([128, 512], f32d, tag="mm")
                    nc.tensor.matmul(ssq[:], onesblk_t[:], q2[:], start=True, stop=True)
                    # rstd = (ms+eps)^-1/2 via ln+exp: stays in the exp act
                    # table, so the whole kernel runs off one table (no
                    # ACT_TABLE_LOAD thrash between Sqrt and Exp)
                    sd = qwork.tile([128, 512], f32d, tag="sd")
                    nc.scalar.activation(out=sd[:], in_=ssq[:], func=AF.Ln,
                                         bias=eps_t[:, :], scale=1.0 / HD)
                    nc.scalar.activation(out=rstd[:, sl], in_=sd[:], func=AF.Exp,
                                         scale=-0.5)
                # rope on full (128, S1Q) rows
                qs = qwork.tile([128, S1Q], bfd, tag="qs")
                nc.vector.stream_shuffle(out=qs[:], in_=qt[:], mask=swap_mask)
                t1 = qwork.tile([128, S1Q], bfd, tag="t1")
                nc.vector.tensor_mul(t1[:], qt[:], ct_t[:, qoff:qoff + S1Q])
                t2 = qwork.tile([128, S1Q], bfd, tag="t2")
                nc.gpsimd.tensor_mul(t2[:], qs[:], st_t[:, qoff:qoff + S1Q])
                core = qwork.tile([128, S1Q], bfd, tag="core")
                nc.vector.tensor_add(core[:], t1[:], t2[:])
                for n in range(2):
                    sl = slice(512 * n, 512 * (n + 1))
                    nc.gpsimd.tensor_mul(qr_t[:, m, sl], core[:, sl], rstd[:, sl])

        def attn_phase(qq):
            for j in range(8):          # head pairs: heads 2j (rows 0:64), 2j+1
                exs = {}
                for t in range(2):
                    for si, tag in ((0, "pscA"), (1, "pscB")):
                        rs = slice(64 * si, 64 * (si + 1))
                        psc = psSc.tile([128, S1Q], f32d, tag=tag)
                        for n in range(2):
                            sl = slice(512 * n, 512 * (n + 1))
                            nc.tensor.matmul(
                                psc[:, sl],
                                kn_t[rs, j, 128 * t:128 * (t + 1)],
                                qr_t[rs, j, sl], start=True, stop=True,
                            )
                        ex = awork.tile([128, S1Q], bfd, tag=f"ex{si}")
                        nc.scalar.activation(out=ex[:], in_=psc[:], func=AF.Exp)
                        exs[(si, t)] = ex
                # attn@v numerators for BOTH heads -> U (col-packed per strip),
                # softmax denominators for both -> D at MATCHING partitions,
                # so the normalize runs full-width with aligned operands.
                for n in range(2):
                    sl = slice(512 * n, 512 * (n + 1))
                    U = psPb.tile([128, 512], f32d, tag="pbU")
                    D = psPb.tile([128, 512], f32d, tag="pbD")
                    for si in range(2):
                        h = 2 * j + si
                        strip = 64 * si
                        rs = slice(strip, strip + 64)
                        for t in range(2):
                            nc.tensor.matmul(
                                U[rs, :], va_t[:, t, h, 0:64], exs[(si, t)][:, sl],
                                start=(t == 0), stop=(t == 1),
                                tile_position=(0, strip),
                            )
                    for si in range(2):
                        h = 2 * j + si
                        strip = 64 * si
                        rs = slice(strip, strip + 64)
                        for t in range(2):
                            nc.tensor.matmul(
                                D[rs, :], va_t[:, t, h, 64:128], exs[(si, t)][:, sl],
                                start=(t == 0), stop=(t == 1),
                                tile_position=(0, strip),
                            )
                    rcp = awork.tile([128, 512], f32d, tag="rcp")
                    nc.vector.reciprocal_approx_fast(out=rcp[:], in_=D[:])
                    nc.vector.tensor_mul(aT_t[:, j, sl], U[:], rcp[:])

        def o_phase(qq):
            qoff = qq * S1Q
            for mo in range(8):
                osb = owork.tile([128, QDIM], f32d, tag="osb")
                for n in range(2):
                    sl = slice(512 * n, 512 * (n + 1))
                    pso = psMain.tile([128, 512], f32d, tag="mm")
                    for k in range(8):
                        nc.tensor.matmul(
                            pso[:], aT_t[:, k, 128 * mo:128 * (mo + 1)],
                            wo_t[:, k, sl], start=(k == 0), stop=False,
                        )
                    nc.tensor.matmul(pso[:], ones1_t[:], bo_t[:, sl],
                                     start=False, stop=True)
                    nc.vector.tensor_copy(out=osb[:, sl], in_=pso[:])
                nc.sync.dma_start(
                    out=out_d[qoff + 128 * mo:qoff + 128 * (mo + 1), :], in_=osb[:]
                )

        # ---- quarter 0 projections first so PE starts early
        q_phase(0)

        # ================= KV phase =================
        with tc.tile_pool(name="kvw", bufs=1) as kvw, \
             tc.tile_pool(name="kvs", bufs=2) as kvs, \
             tc.tile_pool(name="wvs", bufs=3) as wvs:
            yt_t = kvw.tile([128, 16, S2], bfd)
            for k in range(16):
                nc.sync.dma_start_transpose(
                    out=yt_t[:, k, :], in_=ybf[:, 128 * k:128 * (k + 1)]
                )
            # K projection: per m, stream this m's 128-col slice of WkT
            for m in range(8):
                wkm = kvs.tile([128, 16, 128], bfd, tag="wkm")
                nc.sync.dma_start(
                    out=wkm[:],
                    in_=wkT_d[:, 128 * m:128 * (m + 1)].rearrange(
                        "(t p) c -> p t c", p=128
                    ),
                )
                ps = psMain.tile([128, S2], f32d, tag="mm")
                for k in range(16):
                    nc.tensor.matmul(ps[:], wkm[:, k, :], yt_t[:, k, :],
                                     start=(k == 0), stop=(k == 15))
                ktb = kvs.tile([128, S2], bfd, tag="ktb")
                nc.scalar.activation(out=ktb[:], in_=ps[:], func=AF.Identity,
                                     bias=bk8[:, m:m + 1], scale=1.0)
                ksq = kvs.tile([128, S2], bfd, tag="ksq")
                nc.vector.tensor_mul(ksq[:], ktb[:], ktb[:])
                ssk = psMain.tile([128, S2], f32d, tag="mm")
                nc.tensor.matmul(ssk[:], onesblk_t[:], ksq[:], start=True, stop=True)
                sdk = kvs.tile([128, S2], f32d, tag="sdk")
                nc.scalar.activation(out=sdk[:], in_=ssk[:], func=AF.Ln,
                                     bias=eps_t[:, :], scale=1.0 / HD)
                rstdk = kvs.tile([128, S2], f32d, tag="rstdk")
                nc.scalar.activation(out=rstdk[:], in_=sdk[:], func=AF.Exp,
                                     scale=-0.5)
                nc.vector.scalar_tensor_tensor(
                    out=kn_t[:, m, :], in0=ktb[:], scalar=gk_t[:, 0:1], in1=rstdk[:],
                    op0=AO.mult, op1=AO.mult,
                )
            # V projection (natural layout): psV accumulators on the psc banks
            psv0 = psSc.tile([128, QDIM], f32d, tag="pscA")
            psv1 = psSc.tile([128, QDIM], f32d, tag="pscB")
            psvs = (psv0, psv1)
            for k in range(16):
                wv_k = wvs.tile([128, QDIM], bfd, tag="wv")
                nc.sync.dma_start(out=wv_k[:], in_=wvT_d[128 * k:128 * (k + 1), :])
                for mt in range(2):
                    for n in range(2):
                        sl = slice(512 * n, 512 * (n + 1))
                        nc.tensor.matmul(
                            psvs[mt][:, sl], yt_t[:, k, 128 * mt:128 * (mt + 1)],
                            wv_k[:, sl], start=(k == 0), stop=False,
                        )
            for mt in range(2):
                for n in range(2):
                    sl = slice(512 * n, 512 * (n + 1))
                    nc.tensor.matmul(psvs[mt][:, sl], ones1_t[:], bv_t[:, sl],
                                     start=False, stop=True)
                vbf = kvs.tile([128, QDIM], bfd, tag="vbf")
                nc.vector.tensor_copy(out=vbf[:], in_=psvs[mt][:])
                nc.vector.tensor_copy(
                    out=va_t[:, mt, :, 0:64],
                    in_=vbf.rearrange("p (h d) -> p h d", h=NH),
                )
                nc.vector.memset(va_t[:, mt, :, 64:128], 1.0)

        # ---- wo loads late (O phase needs them only ~100us in)
        nc.sync.dma_start(
            out=wo_t[:], in_=woT_d.rearrange("(t p) c -> p t c", p=128)
        )

        attn_phase(0)
        o_phase(0)

        for qq in range(1, NQ):
            qoff = qq * S1Q
            for k in range(8):
                nc.sync.dma_start_transpose(
                    out=xt_t[:, k, :], in_=xbf[qoff:qoff + S1Q, 128 * k:128 * (k + 1)]
                )
            q_phase(qq)
            attn_phase(qq)
            o_phase(qq)

    nc.compile()
    return nc


def _get_program():
    global _PROGRAM
    if _PROGRAM is None:
        _PROGRAM = _build_program()
    return _PROGRAM


# ------------------------------------------------------------------- kernel
def kernel(x, y, wq, bq, wkv, bkv, wo, bo, qn_g, qn_b, kn_g, kn_b):
    global last_exec_time_ns, last_trace_path
    from concourse.bass_utils import run_bass_kernel_spmd

    t = _host_tables(wq, bq, wkv, bkv, wo, bo, qn_g, qn_b, kn_g, kn_b)
    x = _f32(x)
    y = _f32(y)
    nc = _get_program()
    in_maps = []
    for c in range(B):
        m = dict(t)
        m["x"] = np.ascontiguousarray(x[c])
        m["y"] = np.ascontiguousarray(y[c])
        in_maps.append(m)
    trace = bool(int(os.environ.get("KERNEL_TRACE", "0")))
    res = run_bass_kernel_spmd(nc, in_maps, core_ids=list(range(B)), trace=trace)
    last_exec_time_ns = res.exec_time_ns
    if res.instructions_and_trace is not None:
        last_trace_path = res.instructions_and_trace[1]
    return np.stack([res.results[c]["out"] for c in range(B)]).astype(np.float32)


# revision 21
# speedup vs baseline: 1.0995x; 1.0995x over previous
"""Trainium2 Bass kernel for nn_CrossAttention2D_ROPE (B=8, S1=4096, S2=256,
QDIM=1024, KDIM=2048, NH=16, HD=64).

Strategy: data-parallel over batch (8 cores, one batch element each). Per core
a fused pipeline in bf16 (fp32 accumulation):

  - Weights host-prepped: transposed to (in, out), per-head row mean-centering
    folded into wq/wk, RoPE expressed as elementwise C/S tables in q^T layout
    with softmax scale and q-gain folded in.
  - Projections on PE with contraction on partitions (x^T/y^T via bf16 cast +
    DMA transpose). Startup DMAs spread across the sync/scalar/vector/gpsimd
    queues so compute starts ~30us in instead of ~100us.
  - RMS-norm: sum-of-squares via ONE block-diagonal ones matmul whose output
    is already replicated across each head's 64 partitions (fp32-exact), so
    sqrt/reciprocal run at full width and no fp32 broadcast matmul is needed.
  - RoPE: qr = (q*C + pairswap(q)*S) * rstd; pairswap via DVE stream_shuffle;
    elementwise work split across DVE and GpSimd.
  - Attention per head pair: score matmuls (K=64) for the two heads of a pair
    are packed into disjoint PE row groups (concurrent); exp on ScalarE out of
    PSUM; one M=128 [v|ones] matmul produces attn@v (rows 0:64) and the
    softmax denominator (rows 64:128) in a single pass; normalize via a
    cross-partition-base reciprocal+multiply on DVE.
  - Output projection in natural layout, bias via K=1 ones-row matmuls.
"""

import os
import numpy as np
import ml_dtypes

BF = ml_dtypes.bfloat16
QDIM, KDIM, NH, HD = 1024, 2048, 16, 64
H, W, B, S2 = 64, 64, 8, 256
S1 = H * W
EPS = 1e-6
SCALE = HD ** -0.5
NQ = 4            # process s1 in quarters
S1Q = S1 // NQ    # 1024

last_exec_time_ns = None
last_trace_path = None


# ----------------------------------------------------------------- host prep
def _bf16(a):
    return np.asarray(a, np.float32).astype(BF)


def _f32(a):
    return np.asarray(a, np.float32)


def _center_rows_per_head(w):
    out = _f32(w).copy()
    for _ in range(3):
        wb = _f32(_bf16(out))
        resid = wb.reshape(NH, HD, -1).mean(axis=1, keepdims=True)
        out = wb - np.broadcast_to(resid, (NH, HD, wb.shape[-1])).reshape(wb.shape)
    return _bf16(out)


def _center_bias_per_head(b):
    b = _f32(b)
    return b - np.repeat(b.reshape(NH, HD).mean(axis=1), HD)


def _rope_tables(qn_g):
    d4 = HD // 4
    inv = 1.0 / (10000.0 ** (np.arange(0, HD // 2, 2, dtype=np.float32) / (HD // 2)))
    fh = np.outer(np.arange(H, dtype=np.float32), inv)
    fw = np.outer(np.arange(W, dtype=np.float32), inv)
    ang = np.stack([
        np.broadcast_to(fh[:, None, :], (H, W, d4)),
        np.broadcast_to(fw[None, :, :], (H, W, d4)),
    ], axis=-1).reshape(S1, HD // 2)
    cos = np.cos(ang)
    sin = np.sin(ang)
    g = _f32(qn_g)
    C = np.zeros((HD, S1), np.float32)
    S = np.zeros((HD, S1), np.float32)
    for i in range(HD // 2):
        C[2 * i] = g[2 * i] * cos[:, i] * SCALE
        C[2 * i + 1] = g[2 * i + 1] * cos[:, i] * SCALE
        S[2 * i] = -g[2 * i + 1] * sin[:, i] * SCALE
        S[2 * i + 1] = g[2 * i] * sin[:, i] * SCALE
    return _bf16(np.concatenate([C, C], 0)), _bf16(np.concatenate([S, S], 0))


def _host_tables(wq, bq, wkv, bkv, wo, bo, qn_g, qn_b, kn_g, kn_b):
    assert not np.any(_f32(qn_b)) and not np.any(_f32(kn_b)), \
        "nonzero layernorm beta not implemented"
    t = {}
    t["WqT"] = np.ascontiguousarray(_center_rows_per_head(wq).T)
    t["bq_c"] = _center_bias_per_head(bq).reshape(QDIM, 1)
    t["WkT"] = np.ascontiguousarray(_center_rows_per_head(_f32(wkv)[0:QDIM]).T)
    t["bk_c"] = _center_bias_per_head(_f32(bkv)[0:QDIM]).reshape(QDIM, 1)
    t["WvT"] = np.ascontiguousarray(_bf16(_f32(wkv)[QDIM:]).T)
    t["bv"] = _bf16(_f32(bkv)[QDIM:]).reshape(1, QDIM)
    t["WoT"] = np.ascontiguousarray(_bf16(wo).T)
    t["bo"] = _bf16(bo).reshape(1, QDIM)
    t["CT"], t["ST"] = _rope_tables(qn_g)
    t["gk_col"] = np.tile(_f32(kn_g), 2).reshape(128, 1)
    # block-diagonal ones: col m sums the 64-partition strip containing m,
    # so lhsT=onesblk replicates each strip's partition-sum across the strip
    blk = np.zeros((128, 128), np.float32)
    blk[0:64, 0:64] = 1.0
    blk[64:128, 64:128] = 1.0
    t["onesblk"] = _bf16(blk)
    t["ones1"] = _bf16(np.ones((1, 128), np.float32))
    return t


# ------------------------------------------------------------- bass program
_PROGRAM = None


def _build_program():
    import concourse.bass as bass
    import concourse.bacc as bacc
    import concourse.mybir as mybir
    import concourse.tile as tile
    from contextlib import ExitStack

    bfd = mybir.dt.bfloat16
    f32d = mybir.dt.float32
    AF = mybir.ActivationFunctionType
    AO = mybir.AluOpType

    nc = bacc.Bacc("TRN2", target_bir_lowering=False, debug=False)

    def din(name, shape, dt):
        return nc.dram_tensor(name, shape, dt, kind="ExternalInput").ap()

    x_d = din("x", [S1, QDIM], f32d)
    y_d = din("y", [S2, KDIM], f32d)
    wqT_d = din("WqT", [QDIM, QDIM], bfd)
    wkT_d = din("WkT", [KDIM, QDIM], bfd)
    wvT_d = din("WvT", [KDIM, QDIM], bfd)
    woT_d = din("WoT", [QDIM, QDIM], bfd)
    bq_d = din("bq_c", [QDIM, 1], f32d)
    bk_d = din("bk_c", [QDIM, 1], f32d)
    bv_d = din("bv", [1, QDIM], bfd)
    bo_d = din("bo", [1, QDIM], bfd)
    ct_d = din("CT", [128, S1], bfd)
    st_d = din("ST", [128, S1], bfd)
    gk_d = din("gk_col", [128, 1], f32d)
    onesblk_d = din("onesblk", [128, 128], bfd)
    ones1_d = din("ones1", [1, 128], bfd)
    out_d = nc.dram_tensor("out", [S1, QDIM], f32d, kind="ExternalOutput").ap()
    xbf = nc.dram_tensor("xbf", [S1, QDIM], bfd).ap()
    ybf = nc.dram_tensor("ybf", [S2, KDIM], bfd).ap()

    swap_mask = []
    for g in range(16):
        swap_mask += [2 * g + 1, 2 * g]

    with tile.TileContext(nc) as tc, ExitStack() as ctx:
        # Pre-load the one activation table that covers every function this
        # kernel uses (identity/copy/square/ln/exp); without this the
        # auto-inserter ping-pongs between per-function tables (~140 loads
        # at 1.3us each on the scalar engine).
        from concourse.hw_specs import get_activation_tables
        tabs = list(get_activation_tables(nc.m.arch))
        nc.scalar.add_instruction(
            mybir.InstLoadActFuncSet(
                name=nc.get_next_instruction_name(), ins=[], outs=[],
                act_func_set_id=tabs.index("natural_log_exp_and_others"),
            )
        )
        const = ctx.enter_context(tc.tile_pool(name="const", bufs=1))
        persist = ctx.enter_context(tc.tile_pool(name="persist", bufs=1))
        # kernel-wide PSUM plan (8 banks total):
        #   psMain "mm"  2x[128,512] = 2 banks (proj accumulators + norm reps)
        #   psSc  pscA/pscB 1x[128,1024] each = 4 banks (scores; V-proj accums)
        #   psPb  pbA/pbB  1x[128,512] each = 2 banks (attn@v + denominator)
        psMain = ctx.enter_context(tc.tile_pool(name="psMain", bufs=2, space="PSUM"))
        psSc = ctx.enter_context(tc.tile_pool(name="psSc", bufs=1, space="PSUM"))
        psPb = ctx.enter_context(tc.tile_pool(name="psPb", bufs=1, space="PSUM"))
        qwork = ctx.enter_context(tc.tile_pool(name="qwork", bufs=2))
        awork = ctx.enter_context(tc.tile_pool(name="awork", bufs=2))
        owork = ctx.enter_context(tc.tile_pool(name="owork", bufs=2))

        # ---- constants (sync queue; small)
        bq8 = const.tile([128, 8], f32d)
        nc.sync.dma_start(out=bq8[:], in_=bq_d.rearrange("(m p) o -> p (m o)", p=128))
        bk8 = const.tile([128, 8], f32d)
        nc.sync.dma_start(out=bk8[:], in_=bk_d.rearrange("(m p) o -> p (m o)", p=128))
        gk_t = const.tile([128, 1], f32d)
        nc.sync.dma_start(out=gk_t[:], in_=gk_d[:])
        bv_t = const.tile([1, QDIM], bfd)
        nc.sync.dma_start(out=bv_t[:], in_=bv_d[:])
        bo_t = const.tile([1, QDIM], bfd)
        nc.sync.dma_start(out=bo_t[:], in_=bo_d[:])
        onesblk_t = const.tile([128, 128], bfd)
        nc.sync.dma_start(out=onesblk_t[:], in_=onesblk_d[:])
        ones1_t = const.tile([1, 128], bfd)
        nc.sync.dma_start(out=ones1_t[:], in_=ones1_d[:])
        ct_t = const.tile([128, S1], bfd)
        nc.sync.dma_start(out=ct_t[:], in_=ct_d[:])
        st_t = const.tile([128, S1], bfd)
        nc.sync.dma_start(out=st_t[:], in_=st_d[:])
        eps_t = const.tile([128, 1], f32d)
        nc.vector.memset(eps_t[:, :], EPS)

        # ---- persistent activations / weights
        kn_t = persist.tile([128, 8, S2], bfd)       # normalized k^T
        va_t = persist.tile([128, 2, NH, 128], bfd)  # [v_h | ones] per s2-tile
        wq_t = persist.tile([128, 8, QDIM], bfd)
        wo_t = persist.tile([128, 8, QDIM], bfd)
        qr_t = persist.tile([128, 8, S1Q], bfd)      # per-quarter roped q^T
        aT_t = persist.tile([128, 8, S1Q], bfd)      # per-quarter attn out^T
        xt_t = persist.tile([128, 8, S1Q], bfd)      # per-quarter x^T

        # ---- dtype casts (SWDGE on the gpsimd queue); quarter 0 first, in
        # token halves so the first projection matmuls can start early
        nc.gpsimd.dma_start(out=xbf[0:512, :], in_=x_d[0:512, :])
        nc.gpsimd.dma_start(out=xbf[512:S1Q, :], in_=x_d[512:S1Q, :])
        nc.gpsimd.dma_start(out=ybf[:], in_=y_d[:])
        for qq in range(1, NQ):
            nc.gpsimd.dma_start(
                out=xbf[qq * S1Q:(qq + 1) * S1Q, :], in_=x_d[qq * S1Q:(qq + 1) * S1Q, :]
            )
        # wq on scalar queue (needed first), then quarter-0 x^T transposes
        nc.scalar.dma_start(
            out=wq_t[:], in_=wqT_d.rearrange("(t p) c -> p t c", p=128)
        )
        for half in range(2):
            hs, he = 512 * half, 512 * (half + 1)
            for k in range(8):
                nc.scalar.dma_start_transpose(
                    out=xt_t[:, k, hs:he], in_=xbf[hs:he, 128 * k:128 * (k + 1)]
                )

        # ================= per-quarter phases =================
        def q_phase(qq):
            qoff = qq * S1Q
            for m in range(8):
                qt = qwork.tile([128, S1Q], bfd, tag="qt")
                rstd = qwork.tile([128, S1Q], f32d, tag="rstd")
                for n in range(2):
                    sl = slice(512 * n, 512 * (n + 1))
                    psq = psMain.tile([128, 512], f32d, tag="mm")
                    for k in range(8):
                        nc.tensor.matmul(
                            psq[:], wq_t[:, k, 128 * m:128 * (m + 1)],
                            xt_t[:, k, sl], start=(k == 0), stop=(k == 7),
                        )
                    nc.scalar.activation(out=qt[:, sl], in_=psq[:], func=AF.Identity,
                                         bias=bq8[:, m:m + 1], scale=1.0)
                    q2 = qwork.tile([128, 512], bfd, tag="q2")
                    nc.vector.tensor_mul(q2[:], qt[:, sl], qt[:, sl])
                    ssq = psMain.tile([128, 512], f32d, tag="mm")
                    nc.tensor.matmul(ssq[:], onesblk_t[:], q2[:], start=True, stop=True)
                    # rstd = (ms+eps)^-1/2 via ln+exp: stays in the exp act
                    # table, so the whole kernel runs off one table (no
                    # ACT_TABLE_LOAD thrash between Sqrt and Exp)
                    sd = qwork.tile([128, 512], f32d, tag="sd")
                    nc.scalar.activation(out=sd[:], in_=ssq[:], func=AF.Ln,
                                         bias=eps_t[:, :], scale=1.0 / HD)
                    nc.scalar.activation(out=rstd[:, sl], in_=sd[:], func=AF.Exp,
                                         scale=-0.5)
                # rope on full (128, S1Q) rows
                qs = qwork.tile([128, S1Q], bfd, tag="qs")
                nc.vector.stream_shuffle(out=qs[:], in_=qt[:], mask=swap_mask)
                t1 = qwork.tile([128, S1Q], bfd, tag="t1")
                nc.vector.tensor_mul(t1[:], qt[:], ct_t[:, qoff:qoff + S1Q])
                t2 = qwork.tile([128, S1Q], bfd, tag="t2")
                nc.gpsimd.tensor_mul(t2[:], qs[:], st_t[:, qoff:qoff + S1Q])
                core = qwork.tile([128, S1Q], bfd, tag="core")
                nc.vector.tensor_add(core[:], t1[:], t2[:])
                for n in range(2):
                    sl = slice(512 * n, 512 * (n + 1))
                    nc.gpsimd.tensor_mul(qr_t[:, m, sl], core[:, sl], rstd[:, sl])

        def attn_phase(qq):
            for j in range(8):          # head pairs: heads 2j (rows 0:64), 2j+1
                exs = {}
                for t in range(2):
                    for si, tag in ((0, "pscA"), (1, "pscB")):
                        rs = slice(64 * si, 64 * (si + 1))
                        psc = psSc.tile([128, S1Q], f32d, tag=tag)
                        for n in range(2):
                            sl = slice(512 * n, 512 * (n + 1))
                            nc.tensor.matmul(
                                psc[:, sl],
                                kn_t[rs, j, 128 * t:128 * (t + 1)],
                                qr_t[rs, j, sl], start=True, stop=True,
                            )
                        ex = awork.tile([128, S1Q], bfd, tag=f"ex{si}")
                        nc.scalar.activation(out=ex[:], in_=psc[:], func=AF.Exp)
                        exs[(si, t)] = ex
                # attn@v numerators for BOTH heads -> U (col-packed per strip),
                # softmax denominators for both -> D at MATCHING partitions,
                # so the normalize runs full-width with aligned operands.
                for n in range(2):
                    sl = slice(512 * n, 512 * (n + 1))
                    U = psPb.tile([128, 512], f32d, tag="pbU")
                    D = psPb.tile([128, 512], f32d, tag="pbD")
                    for si in range(2):
                        h = 2 * j + si
                        strip = 64 * si
                        rs = slice(strip, strip + 64)
                        for t in range(2):
                            nc.tensor.matmul(
                                U[rs, :], va_t[:, t, h, 0:64], exs[(si, t)][:, sl],
                                start=(t == 0), stop=(t == 1),
                                tile_position=(0, strip),
                            )
                    for si in range(2):
                        h = 2 * j + si
                        strip = 64 * si
                        rs = slice(strip, strip + 64)
                        for t in range(2):
                            nc.tensor.matmul(
                                D[rs, :], va_t[:, t, h, 64:128], exs[(si, t)][:, sl],
                                start=(t == 0), stop=(t == 1),
                                tile_position=(0, strip),
                            )
                    rcp = awork.tile([128, 512], f32d, tag="rcp")
                    nc.vector.reciprocal_approx_fast(out=rcp[:], in_=D[:])
                    nc.vector.tensor_mul(aT_t[:, j, sl], U[:], rcp[:])

        def o_phase(qq):
            qoff = qq * S1Q
            for mo in range(8):
                osb = owork.tile([128, QDIM], f32d, tag="osb")
                for n in range(2):
                    sl = slice(512 * n, 512 * (n + 1))
                    pso = psMain.tile([128, 512], f32d, tag="mm")
                    for k in range(8):
                        nc.tensor.matmul(
                            pso[:], aT_t[:, k, 128 * mo:128 * (mo + 1)],
                            wo_t[:, k, sl], start=(k == 0), stop=False,
                        )
                    nc.tensor.matmul(pso[:], ones1_t[:], bo_t[:, sl],
                                     start=False, stop=True)
                    nc.vector.tensor_copy(out=osb[:, sl], in_=pso[:])
                nc.sync.dma_start(
                    out=out_d[qoff + 128 * mo:qoff + 128 * (mo + 1), :], in_=osb[:]
                )

        # ---- quarter 0 projections first so PE starts early
        q_phase(0)

        # ================= KV phase =================
        with tc.tile_pool(name="kvw", bufs=1) as kvw, \
             tc.tile_pool(name="kvs", bufs=2) as kvs, \
             tc.tile_pool(name="wvs", bufs=3) as wvs:
            yt_t = kvw.tile([128, 16, S2], bfd)
            for k in range(16):
                nc.sync.dma_start_transpose(
                    out=yt_t[:, k, :], in_=ybf[:, 128 * k:128 * (k + 1)]
                )
            # K projection: per m, stream this m's 128-col slice of WkT
            for m in range(8):
                wkm = kvs.tile([128, 16, 128], bfd, tag="wkm")
                nc.sync.dma_start(
                    out=wkm[:],
                    in_=wkT_d[:, 128 * m:128 * (m + 1)].rearrange(
                        "(t p) c -> p t c", p=128
                    ),
                )
                ps = psMain.tile([128, S2], f32d, tag="mm")
                for k in range(16):
                    nc.tensor.matmul(ps[:], wkm[:, k, :], yt_t[:, k, :],
                                     start=(k == 0), stop=(k == 15))
                ktb = kvs.tile([128, S2], bfd, tag="ktb")
                nc.scalar.activation(out=ktb[:], in_=ps[:], func=AF.Identity,
                                     bias=bk8[:, m:m + 1], scale=1.0)
                ksq = kvs.tile([128, S2], bfd, tag="ksq")
                nc.vector.tensor_mul(ksq[:], ktb[:], ktb[:])
                ssk = psMain.tile([128, S2], f32d, tag="mm")
                nc.tensor.matmul(ssk[:], onesblk_t[:], ksq[:], start=True, stop=True)
                sdk = kvs.tile([128, S2], f32d, tag="sdk")
                nc.scalar.activation(out=sdk[:], in_=ssk[:], func=AF.Ln,
                                     bias=eps_t[:, :], scale=1.0 / HD)
                rstdk = kvs.tile([128, S2], f32d, tag="rstdk")
                nc.scalar.activation(out=rstdk[:], in_=sdk[:], func=AF.Exp,
                                     scale=-0.5)
                nc.vector.scalar_tensor_tensor(
                    out=kn_t[:, m, :], in0=ktb[:], scalar=gk_t[:, 0:1], in1=rstdk[:],
                    op0=AO.mult, op1=AO.mult,
                )
            # V projection (natural layout): psV accumulators on the psc banks
            psv0 = psSc.tile([128, QDIM], f32d, tag="pscA")
            psv1 = psSc.tile([128, QDIM], f32d, tag="pscB")
            psvs = (psv0, psv1)
            for k in range(16):
                wv_k = wvs.tile([128, QDIM], bfd, tag="wv")
                nc.sync.dma_start(out=wv_k[:], in_=wvT_d[128 * k:128 * (k + 1), :])
                for mt in range(2):
                    for n in range(2):
                        sl = slice(512 * n, 512 * (n + 1))
                        nc.tensor.matmul(
                            psvs[mt][:, sl], yt_t[:, k, 128 * mt:128 * (mt + 1)],
                            wv_k[:, sl], start=(k == 0), stop=False,
                        )
            for mt in range(2):
                for n in range(2):
                    sl = slice(512 * n, 512 * (n + 1))
                    nc.tensor.matmul(psvs[mt][:, sl], ones1_t[:], bv_t[:, sl],
                                     start=False, stop=True)
                vbf = kvs.tile([128, QDIM], bfd, tag="vbf")
                nc.vector.tensor_copy(out=vbf[:], in_=psvs[mt][:])
                nc.vector.tensor_copy(
                    out=va_t[:, mt, :, 0:64],
                    in_=vbf.rearrange("p (h d) -> p h d", h=NH),
                )
                nc.vector.memset(va_t[:, mt, :, 64:128], 1.0)

        # ---- wo loads late (O phase needs them only ~100us in)
        nc.sync.dma_start(
            out=wo_t[:], in_=woT_d.rearrange("(t p) c -> p t c", p=128)
        )

        attn_phase(0)
        o_phase(0)

        for qq in range(1, NQ):
            qoff = qq * S1Q
            for k in range(8):
                nc.sync.dma_start_transpose(
                    out=xt_t[:, k, :], in_=xbf[qoff:qoff + S1Q, 128 * k:128 * (k + 1)]
                )
            q_phase(qq)
            attn_phase(qq)
            o_phase(qq)

    nc.compile()
    return nc


def _get_program():
    global _PROGRAM
    if _PROGRAM is None:
        _PROGRAM = _build_program()
    return _PROGRAM


# ------------------------------------------------------------------- kernel
def kernel(x, y, wq, bq, wkv, bkv, wo, bo, qn_g, qn_b, kn_g, kn_b):
    global last_exec_time_ns, last_trace_path
    from concourse.bass_utils import run_bass_kernel_spmd

    t = _host_tables(wq, bq, wkv, bkv, wo, bo, qn_g, qn_b, kn_g, kn_b)
    x = _f32(x)
    y = _f32(y)
    nc = _get_program()
    in_maps = []
    for c in range(B):
        m = dict(t)
        m["x"] = np.ascontiguousarray(x[c])
        m["y"] = np.ascontiguousarray(y[c])
        in_maps.append(m)
    trace = bool(int(os.environ.get("KERNEL_TRACE", "0")))
    res = run_bass_kernel_spmd(nc, in_maps, core_ids=list(range(B)), trace=trace)
    last_exec_time_ns = res.exec_time_ns
    if res.instructions_and_trace is not None:
        last_trace_path = res.instructions_and_trace[1]
    return np.stack([res.results[c]["out"] for c in range(B)]).astype(np.float32)
